# revision 25
# baseline (speedup 1.0000x reference)
"""Trainium2 Bass kernel for batched self-attention with input projections.

Problem: B=8, N=2048, D=131
    Q = q @ Wq.T + bq;  K = k @ Wk.T + bk;  V = v @ Wv.T + bv
    out = softmax(Q K^T / sqrt(131)) V

One batch element per NeuronCore (8 cores, no communication).

Host prep (layout/algebra only):
  - Tokens augmented with a ones-row: X = [x^T; 1] in [132, 2048] so biases
    fold into the projection matmuls.
  - Scores: Q K^T = Xq (Wq'^T Wk'/sqrt(D)) Xk^T = Xq G Xk^T, G [132,132].
    SVD-truncate G to rank 128 (exact rank 131; error ~2e-5) so the big S
    matmul is a single K=128 contraction:  S = (Xq Aq)(Xk Ak)^T.
  - Value path: W2 [132,132] maps X -> [V | 1] (bias row + denominator
    ones-column).  SVD-truncate W2 = L R^T to rank 128 so the O-matmul
    contracts into a 128-wide latent:  O' = (P Xv L) R^T, with O'[:,131]
    the softmax denominator.
  - Projections/S in bf16 (fp32 PSUM accumulation); the O path (exp
    weights E and the value latent VL) in fp8e4m3 so the O accumulation
    runs in DoubleRow perf mode: each matmul contracts TWO 128-token
    k-tiles at once (lhsT [128,2,128], rhs [128,2,1024]), halving the PE
    time of the biggest accumulation.  Measured rel err ~1.05e-2 (numpy
    bit-accurate sim of the dtype pipeline).  |S| < 3 so softmax without
    max-subtraction is safe.

Per core:
  QT[e',n] = Aq^T Xq, KT[e',n] = Ak^T Xk      (hi[128]+lo[4] d-chunks)
  VL[n,l]  = Xv^T L -> fp8 pair tiles [128,2,128] (j, j+1)
  for i-half h (1024 cols), j-block (16):
      ST = KT_j^T QT_h  (one [128,1024] matmul) -> exp on ACT -> fp8 E
      pair tiles [128,2,1024]; every 2 j's one DoubleRow matmul
      accumulates Ohat^T[l, h] in PSUM.
  O'[i,132] = Ohat_i R^T;  out = O'[:,0:131] / O'[:,131]

ACT does exp only (the activation table preloads via a dummy exp during
the DMA window); DVE does all PSUM->SBUF copies and the normalize; a
burst of junk matmuls during the input DMA warms the PE clock gate; a
post-finalize pass drops Ldweights instructions that reload identical
weights.
"""

import numpy as np
import ml_dtypes

P = 128          # partitions / PE width
N = 2048         # tokens per core
D = 131          # embed dim
DP = 132         # embed dim + ones row
DLO = DP - P     # tail contraction rows (4)
R = 128          # truncated rank (QK interaction and V latent)
EV = 132         # final output cols (131 + denominator)
NB = N // P      # 16 token blocks
NPAIR = NB // 2  # 8 j-block pairs (DoubleRow granule)
HW = 1024        # i-half width
NH = N // HW     # 2 halves
NCORES = 8

QOFF, KOFF, VOFF = 0, N, 2 * N          # column offsets in packed xall
AQOFF, AKOFF, LOFF = 0, R, 2 * R        # column offsets in packed weights

_BF16 = ml_dtypes.bfloat16


def build_nc():
    """Build the single-core Bass graph (same NEFF runs SPMD on all 8 cores)."""
    from contextlib import ExitStack

    import concourse.bacc as bacc
    import concourse.mybir as mybir
    import concourse.tile as tile
    from concourse.bass import ts

    bf = mybir.dt.bfloat16
    f8 = mybir.dt.float8e4
    f32 = mybir.dt.float32
    EXP = mybir.ActivationFunctionType.Exp
    COPY = mybir.ActivationFunctionType.Copy
    DR = mybir.MatmulPerfMode.DoubleRow

    nc = bacc.Bacc()
    xall = nc.declare_dram_parameter("xall", [DP, 3 * N], bf, isOutput=False)
    wpack = nc.declare_dram_parameter("wpack", [DP, 3 * R], bf, isOutput=False)
    rmat = nc.declare_dram_parameter("rmat", [R, EV], bf, isOutput=False)
    out = nc.declare_dram_parameter("out", [N, D], f32, isOutput=True)

    with tile.TileContext(nc) as tc, ExitStack() as ctx:
        const = ctx.enter_context(tc.tile_pool(name="const", bufs=1))
        xin = ctx.enter_context(tc.tile_pool(name="xin", bufs=1))
        proj = ctx.enter_context(tc.tile_pool(name="proj", bufs=1))
        vpool = ctx.enter_context(tc.tile_pool(name="vpool", bufs=1))
        epool = ctx.enter_context(tc.tile_pool(name="epool", bufs=6))
        ebp = ctx.enter_context(tc.tile_pool(name="ebp", bufs=4))
        ohs = ctx.enter_context(tc.tile_pool(name="ohs", bufs=1))
        outp = ctx.enter_context(tc.tile_pool(name="outp", bufs=4))
        warm = ctx.enter_context(tc.tile_pool(name="warm", bufs=1))
        # PSUM budget (8 banks): proj/final 2 x [128,512] = 2, scores
        # 2 x [128,1024] = 4, Ohat accumulator 1 x [128,1024] = 2.
        psp = ctx.enter_context(tc.tile_pool(name="psp", bufs=2, space="PSUM"))
        psst = ctx.enter_context(tc.tile_pool(name="psst", bufs=2, space="PSUM"))
        psoh = ctx.enter_context(tc.tile_pool(name="psoh", bufs=1, space="PSUM"))

        # ---- DMA loads.  Column-chunked [128, 1024] transfers (one 2D
        # descriptor each, 2KB per partition line).  Each dma_start costs
        # ~650ns of issue time on its engine's queue, so the issues are
        # spread across engines: sync takes the critical-path Q/K/V order,
        # gpsimd (otherwise idle) takes the weights + tail rows, vector
        # takes one K chunk after its memset.
        wp_hi = const.tile([P, 3 * R], bf)
        wp_lo = const.tile([DLO, 3 * R], bf)
        xall_hi = xin.tile([P, 3 * N], bf)
        xall_lo = xin.tile([DLO, 3 * N], bf)
        rmat_s = const.tile([R, EV], bf)
        # Transfer priority: per-partition DMA write bw is ~1.2-2GB/s, so
        # what matters is bytes-per-partition ahead of each need.  The
        # 4-partition tail rows are extra slow and serialize with each
        # other, so they are split per-half and fronted.  sync carries the
        # 128-partition chunks in first-use order; scalar (idle pre-stream)
        # carries the weights + tail rows.
        nc.sync.dma_start(
            out=xall_hi[:, QOFF:QOFF + HW], in_=xall[0:P, QOFF:QOFF + HW]
        )
        nc.sync.dma_start(
            out=xall_hi[:, KOFF:KOFF + 512], in_=xall[0:P, KOFF:KOFF + 512]
        )
        nc.sync.dma_start(
            out=xall_hi[:, KOFF + 512:KOFF + HW],
            in_=xall[0:P, KOFF + 512:KOFF + HW],
        )
        for lo, hi in (
            (VOFF, VOFF + HW),           # V blocks 0-7
            (VOFF + HW, VOFF + N),       # V blocks 8-15
            (QOFF + HW, QOFF + N),       # Q half 1
        ):
            nc.sync.dma_start(
                out=xall_hi[:, lo:hi], in_=xall[0:P, lo:hi]
            )
        wsrc = warm.tile([P, 512], bf)
        nc.vector.memset(wsrc, 0)
        wdum = warm.tile([P, 4], bf)
        nc.scalar.dma_start(out=wp_hi, in_=wpack[0:P, :])
        nc.scalar.dma_start(out=wp_lo, in_=wpack[P:DP, :])
        nc.scalar.dma_start(
            out=xall_lo[:, QOFF:QOFF + HW], in_=xall[P:DP, QOFF:QOFF + HW]
        )
        nc.scalar.dma_start(
            out=xall_lo[:, KOFF:KOFF + HW], in_=xall[P:DP, KOFF:KOFF + HW]
        )
        nc.scalar.activation(wdum, wsrc[:, 0:4], EXP)
        nc.scalar.dma_start(
            out=xall_lo[:, VOFF:VOFF + HW], in_=xall[P:DP, VOFF:VOFF + HW]
        )
        nc.scalar.dma_start(
            out=xall_lo[:, KOFF + HW:KOFF + N], in_=xall[P:DP, KOFF + HW:KOFF + N]
        )
        nc.scalar.dma_start(
            out=xall_lo[:, VOFF + HW:VOFF + N], in_=xall[P:DP, VOFF + HW:VOFF + N]
        )
        nc.scalar.dma_start(
            out=xall_lo[:, QOFF + HW:QOFF + N], in_=xall[P:DP, QOFF + HW:QOFF + N]
        )
        nc.scalar.dma_start(
            out=xall_hi[:, KOFF + HW:KOFF + N], in_=xall[0:P, KOFF + HW:KOFF + N]
        )
        nc.scalar.dma_start(out=rmat_s, in_=rmat[:, :])

        # ---- junk-matmul burst: keeps the PE clock ramping while the
        # input DMAs land (more junk is interleaved into the projection
        # chain below to fill its DMA-wait bubbles).
        for w in range(3):
            pw = psst.tile([P, HW], f32, tag="pst", name="pw")
            nc.tensor.matmul(pw[:, 0:512], wsrc[:, 0:P], wsrc, start=True, stop=True)

        def junk(n=1):
            for w in range(n):
                pw = psst.tile([P, HW], f32, tag="pst", name="pwj")
                nc.tensor.matmul(
                    pw[:, 0:512], wsrc[:, 0:P], wsrc, start=True, stop=True
                )

        # ---- projection tiles.  QT merged per half so each S_j is ONE
        # [128,1024] matmul; KT chunked [128,512]; VL as fp8 pair tiles.
        qth = [proj.tile([P, HW], bf, tag=f"qh{h}", name=f"qh{h}") for h in range(NH)]
        kts = [proj.tile([P, 512], bf, tag=f"kt{c}", name=f"kt{c}") for c in range(4)]
        vps = [vpool.tile([P, 2, P], f8, tag=f"vp{g}", name=f"vp{g}")
               for g in range(NPAIR)]

        def qk_chunk(dst, woff, xoff, c, on_psst=False, jmid=0):
            if on_psst:
                ppt = psst.tile([P, HW], f32, tag="pst", name="ppk")
                pp = ppt[:, 0:512]
            else:
                pp = psp.tile([P, 512], f32, tag="pp", name="pp")
            nc.tensor.matmul(
                pp,
                wp_hi[:, woff:woff + R],
                xall_hi[:, xoff + c * 512: xoff + (c + 1) * 512],
                start=True,
                stop=False,
            )
            junk(jmid)
            nc.tensor.matmul(
                pp,
                wp_lo[:, woff:woff + R],
                xall_lo[:, xoff + c * 512: xoff + (c + 1) * 512],
                start=False,
                stop=True,
            )
            nc.vector.tensor_copy(dst, pp)

        def vl_group(g4):
            """Project VL for j = 4*g4 .. 4*g4+3 into fp8 pair tiles."""
            pv = psp.tile([P, 512], f32, tag="pp", name="pv")
            for t in range(4):
                j = 4 * g4 + t
                nc.tensor.matmul(
                    pv[:, ts(t, P)],
                    xall_hi[:, VOFF + j * P: VOFF + (j + 1) * P],
                    wp_hi[:, LOFF:LOFF + R],
                    start=True,
                    stop=False,
                )
                nc.tensor.matmul(
                    pv[:, ts(t, P)],
                    xall_lo[:, VOFF + j * P: VOFF + (j + 1) * P],
                    wp_lo[:, LOFF:LOFF + R],
                    start=False,
                    stop=True,
                )
            for t in range(2):
                g = 2 * g4 + t
                nc.vector.tensor_copy(vps[g][:, 0, :], pv[:, ts(2 * t, P)])
                nc.vector.tensor_copy(vps[g][:, 1, :], pv[:, ts(2 * t + 1, P)])

        def s_exp(h, j, edst):
            """S^T_j for half h (512-col matmuls: PSUM-bank limit); exp on
            ACT to bf16 (fp8 output costs ACT +130ns/tile), then the idle
            GpSimd engine casts bf16 -> fp8 for the DoubleRow O matmul."""
            pst = psst.tile([P, HW], f32, tag="pst", name="pst")
            for c in range(2):
                nc.tensor.matmul(
                    pst[:, ts(c, 512)],
                    kts[j // 4][:, ts(j % 4, P)],
                    qth[h][:, ts(c, 512)],
                    start=True,
                    stop=True,
                )
            ebf = ebp.tile([P, HW], bf, tag="eb", name="eb")
            nc.scalar.activation(ebf, pst, EXP)
            nc.gpsimd.tensor_copy(edst, ebf)

        def o_pair(poh, g, ep):
            """DoubleRow fp8 matmuls: contract j-blocks 2g and 2g+1 at once."""
            for c in range(2):
                nc.tensor.matmul(
                    poh[:, ts(c, 512)],
                    vps[g],
                    ep[:, :, ts(c, 512)],
                    start=(g == 0),
                    stop=(g == NPAIR - 1),
                    perf_mode=DR,
                )

        def finalize_group(h, g, ohat, act_mul=False):
            """Two i-blocks -> O' = Ohat R^T, normalize, DMA out.  act_mul
            puts one of the two normalizes on ACT and the output DMA issue
            on the scalar queue (post-exp-stream only)."""
            stage = outp.tile([P, 2, D], f32, tag="stage", name="stage")
            for t in range(2):
                i = 2 * g + t
                po = psp.tile([P, EV], f32, tag="pp", name="po")
                nc.tensor.matmul(
                    po, ohat[:, ts(i % 8, P)], rmat_s, start=True, stop=True
                )
                rec = outp.tile([P, 1], f32, tag="rec", name="rec")
                nc.vector.reciprocal(rec, po[:, D:D + 1])
                if act_mul and t == 1:
                    nc.scalar.activation(
                        stage[:, t, :], po[:, 0:D], COPY, scale=rec
                    )
                else:
                    nc.vector.tensor_scalar_mul(stage[:, t, :], po[:, 0:D], rec)
            nc.sync.dma_start(
                out=out[g * 256:(g + 1) * 256, :].rearrange(
                    "(t p) e -> p t e", p=P
                ),
                in_=stage,
            )

        # ---- h=0 stream: project what each j needs just in time, start
        # the exp stream as early as possible, trail it with the DoubleRow
        # O accumulation; VL groups fill PE slack between S matmuls.
        qk_chunk(qth[0][:, 0:512], AQOFF, QOFF, 0, jmid=1)
        qk_chunk(qth[0][:, 512:HW], AQOFF, QOFF, 1)
        qk_chunk(kts[0], AKOFF, KOFF, 0, on_psst=True, jmid=1)

        poh0 = psoh.tile([P, HW], f32, tag="poh", name="poh0")
        eps0 = []

        def h0_step(j):
            if j % 2 == 0:
                ep = epool.tile([P, 2, HW], f8, tag="e", name=f"e0_{j // 2}")
                eps0.append(ep)
            s_exp(0, j, eps0[j // 2][:, j % 2, :])

        h0_step(0)
        h0_step(1)
        qk_chunk(kts[1], AKOFF, KOFF, 1)
        h0_step(2)
        h0_step(3)
        vl_group(0)
        h0_step(4)
        o_pair(poh0, 0, eps0[0])
        h0_step(5)
        qk_chunk(kts[2], AKOFF, KOFF, 2)
        h0_step(6)
        o_pair(poh0, 1, eps0[1])
        vl_group(1)
        h0_step(7)
        qk_chunk(kts[3], AKOFF, KOFF, 3)
        h0_step(8)
        o_pair(poh0, 2, eps0[2])
        h0_step(9)
        vl_group(2)
        h0_step(10)
        o_pair(poh0, 3, eps0[3])
        h0_step(11)
        vl_group(3)
        h0_step(12)
        o_pair(poh0, 4, eps0[4])
        h0_step(13)
        qk_chunk(qth[1][:, 0:512], AQOFF, QOFF, 2)
        qk_chunk(qth[1][:, 512:HW], AQOFF, QOFF, 3)
        h0_step(14)
        o_pair(poh0, 5, eps0[5])
        h0_step(15)
        o_pair(poh0, 6, eps0[6])
        o_pair(poh0, 7, eps0[7])
        ohat0 = ohs.tile([P, HW], bf, tag="oh0", name="oh0")
        nc.vector.tensor_copy(ohat0, poh0)

        # ---- h=1 stream with h=0 finalization interleaved.
        poh1 = psoh.tile([P, HW], f32, tag="poh", name="poh1")
        eps1 = []

        def h1_step(j):
            if j % 2 == 0:
                ep = epool.tile([P, 2, HW], f8, tag="e", name=f"e1_{j // 2}")
                eps1.append(ep)
            s_exp(1, j, eps1[j // 2][:, j % 2, :])

        for j in range(NB):
            h1_step(j)
            if j % 2 == 1:
                g = j // 2
                o_pair(poh1, g, eps1[g])
                if g < 4:
                    # h0 finals run mid-stream where DVE is idle
                    finalize_group(0, g, ohat0)
        ohat1 = ohs.tile([P, HW], bf, tag="oh1", name="oh1")
        # ACT is free once the exp stream ends; copy in 256-col pieces
        # alternating ACT/DVE so finalize group g can start on piece g
        for p4 in range(4):
            if p4 % 2 == 0:
                nc.scalar.activation(
                    ohat1[:, ts(p4, 256)], poh1[:, ts(p4, 256)], COPY
                )
            else:
                nc.vector.tensor_copy(ohat1[:, ts(p4, 256)], poh1[:, ts(p4, 256)])
            finalize_group(1, 4 + p4, ohat1, act_mul=True)

    return nc


def dedup_ldweights(nc):
    """Drop Ldweights instructions that reload the exact weights already in
    the PE array (same AP, nothing clobbering in between).  The PE keeps the
    stationary operand across matmuls, so a back-to-back identical reload is
    pure dispatch overhead (~107ns each).  Only sync-free Ldweights are
    dropped so semaphore ordering is untouched."""
    dropped = 0
    for f in nc.m.functions:
        for blk in f.blocks:
            insts = list(blk.instructions)
            kept = []
            last_key = None
            for ins in insts:
                tname = type(ins).__name__
                if "PE" in str(getattr(ins, "engine", "")):
                    if tname == "InstLdweights":
                        ap = ins.ins[0]
                        key = (
                            ap.memref,
                            ap.offset,
                            str(ap.ap),
                            str(ap.dtype),
                            str(getattr(ins, "is_transpose", None)),
                        )
                        si = ins.sync_info
                        no_sync = si is None or (
                            len(si.on_wait) == 0 and len(si.on_update) == 0
                        )
                        if key == last_key and no_sync:
                            dropped += 1
                            continue
                        last_key = key
                    elif tname not in (
                        "InstMatmult",
                        "InstEventSemaphore",
                        "InstNoOp",
                        "InstDrain",
                    ):
                        last_key = None
                kept.append(ins)
            if len(kept) != len(insts):
                blk.instructions = kept
    return dropped


def prep_host(query, key, value, Wq, bq, Wk, bk, Wv, bv):
    """Host-side layout/algebra prep. Returns per-core input maps."""
    s = np.sqrt(np.float64(D))
    Wqp = np.concatenate([Wq, bq[:, None]], axis=1)  # [131, 132]
    Wkp = np.concatenate([Wk, bk[:, None]], axis=1)
    G = (Wqp.astype(np.float64).T @ Wkp.astype(np.float64)) / s  # [132, 132]
    U, S, Vt = np.linalg.svd(G)
    Aq = (U[:, :R] * np.sqrt(S[:R])).astype(np.float32)  # [132, 128]
    Ak = (Vt[:R, :].T * np.sqrt(S[:R])).astype(np.float32)

    W2 = np.zeros((DP, EV), np.float64)  # maps X -> [V | 1]
    W2[:D, :D] = Wv.T
    W2[D, :D] = bv
    W2[D, D] = 1.0
    U2, S2, V2t = np.linalg.svd(W2)
    L = (U2[:, :R] * np.sqrt(S2[:R])).astype(np.float32)  # [132, 128]
    Rm = (V2t[:R, :].T * np.sqrt(S2[:R])).astype(np.float32)  # [132, 128]

    wpack = np.concatenate([Aq, Ak, L], axis=1)  # [132, 384]
    wpack16 = np.ascontiguousarray(wpack.astype(_BF16))
    rmat16 = np.ascontiguousarray(Rm.T.astype(_BF16))  # [128, 132]

    ones_row = np.ones((1, N), np.float32)
    in_maps = []
    for c in range(NCORES):
        xs = [np.concatenate([x.T, ones_row], axis=0)
              for x in (query[c], key[c], value[c])]
        xallc = np.concatenate(xs, axis=1)  # [132, 6144]
        in_maps.append({
            "xall": np.ascontiguousarray(xallc.astype(_BF16)),
            "wpack": wpack16,
            "rmat": rmat16,
        })
    return in_maps


_NC_CACHE = {}


def _get_nc():
    if "nc" not in _NC_CACHE:
        nc = build_nc()
        if not nc.is_finalized():
            nc.finalize()  # Bacc.finalize runs the wait-split/EVSEM passes
        dedup_ldweights(nc)
        _NC_CACHE["nc"] = nc
    return _NC_CACHE["nc"]


def run_on_cores(in_maps, trace=False, **kw):
    from concourse.bass_utils import run_bass_kernel_spmd

    nc = _get_nc()
    return run_bass_kernel_spmd(nc, in_maps, core_ids=list(range(NCORES)),
                                trace=trace, **kw)


def kernel(query, key, value, Wq, bq, Wk, bk, Wv, bv):
    in_maps = prep_host(query, key, value, Wq, bq, Wk, bk, Wv, bv)
    res = run_on_cores(in_maps)
    return np.stack([np.asarray(res.results[c]["out"]) for c in range(NCORES)])


# revision 26
# speedup vs baseline: 1.9253x; 1.9253x over previous
"""Trainium2 Bass kernel for batched self-attention with input projections.

Problem: B=8, N=2048, D=131
    Q = q @ Wq.T + bq;  K = k @ Wk.T + bk;  V = v @ Wv.T + bv
    out = softmax(Q K^T / sqrt(131)) V

One batch element per NeuronCore (8 cores, no communication).

Host prep (layout/algebra only):
  - Tokens augmented with a ones-row: X = [x^T; 1] in [132, 2048] so biases
    fold into the projection matmuls.
  - Scores: Q K^T = Xq (Wq'^T Wk'/sqrt(D)) Xk^T = Xq G Xk^T, G [132,132].
    SVD-truncate G to rank 128 (exact rank 131; error ~2e-5) so the big S
    matmul is a single K=128 contraction:  S = (Xq Aq)(Xk Ak)^T.
  - Value path: W2 [132,132] maps X -> [V | 1] (bias row + denominator
    ones-column).  SVD-truncate W2 = L R^T to rank 128 so the O-matmul
    contracts into a 128-wide latent:  O' = (P Xv L) R^T, with O'[:,131]
    the softmax denominator.
  - Projections/S in bf16 (fp32 PSUM accumulation); the O path (exp
    weights E and the value latent VL) in fp8e4m3 so the O accumulation
    runs in DoubleRow perf mode: each matmul contracts TWO 128-token
    k-tiles at once (lhsT [128,2,128], rhs [128,2,1024]), halving the PE
    time of the biggest accumulation.  Measured rel err ~1.05e-2 (numpy
    bit-accurate sim of the dtype pipeline).  |S| < 3 so softmax without
    max-subtraction is safe.

Per core:
  QT[e',n] = Aq^T Xq, KT[e',n] = Ak^T Xk      (hi[128]+lo[4] d-chunks)
  VL[n,l]  = Xv^T L -> fp8 pair tiles [128,2,128] (j, j+1)
  for i-half h (1024 cols), j-block (16):
      ST = KT_j^T QT_h  (one [128,1024] matmul) -> exp on ACT -> fp8 E
      pair tiles [128,2,1024]; every 2 j's one DoubleRow matmul
      accumulates Ohat^T[l, h] in PSUM.
  O'[i,132] = Ohat_i R^T;  out = O'[:,0:131] / O'[:,131]

ACT does exp only (the activation table preloads via a dummy exp during
the DMA window); DVE does all PSUM->SBUF copies and the normalize; a
burst of junk matmuls during the input DMA warms the PE clock gate; a
post-finalize pass drops Ldweights instructions that reload identical
weights.
"""

import numpy as np
import ml_dtypes

P = 128          # partitions / PE width
N = 2048         # tokens per core
D = 131          # embed dim
DP = 132         # embed dim + ones row
DLO = DP - P     # tail contraction rows (4)
R = 128          # truncated rank (QK interaction and V latent)
EV = 132         # final output cols (131 + denominator)
NB = N // P      # 16 token blocks
NPAIR = NB // 2  # 8 j-block pairs (DoubleRow granule)
HW = 1024        # i-half width
NH = N // HW     # 2 halves
NCORES = 8

QOFF, KOFF, VOFF = 0, N, 2 * N          # column offsets in packed xall
AQOFF, AKOFF, LOFF = 0, R, 2 * R        # column offsets in packed weights

_BF16 = ml_dtypes.bfloat16


def build_nc():
    """Build the single-core Bass graph (same NEFF runs SPMD on all 8 cores)."""
    from contextlib import ExitStack

    import concourse.bacc as bacc
    import concourse.mybir as mybir
    import concourse.tile as tile
    from concourse.bass import ts

    bf = mybir.dt.bfloat16
    f8 = mybir.dt.float8e4
    f32 = mybir.dt.float32
    EXP = mybir.ActivationFunctionType.Exp
    COPY = mybir.ActivationFunctionType.Copy
    DR = mybir.MatmulPerfMode.DoubleRow

    nc = bacc.Bacc()
    xall = nc.declare_dram_parameter("xall", [DP, 3 * N], bf, isOutput=False)
    wpack = nc.declare_dram_parameter("wpack", [DP, 3 * R], bf, isOutput=False)
    rmat = nc.declare_dram_parameter("rmat", [R, EV], bf, isOutput=False)
    out = nc.declare_dram_parameter("out", [N, D], f32, isOutput=True)

    with tile.TileContext(nc) as tc, ExitStack() as ctx:
        const = ctx.enter_context(tc.tile_pool(name="const", bufs=1))
        xin = ctx.enter_context(tc.tile_pool(name="xin", bufs=1))
        proj = ctx.enter_context(tc.tile_pool(name="proj", bufs=1))
        vpool = ctx.enter_context(tc.tile_pool(name="vpool", bufs=1))
        epool = ctx.enter_context(tc.tile_pool(name="epool", bufs=6))
        ebp = ctx.enter_context(tc.tile_pool(name="ebp", bufs=4))
        ohs = ctx.enter_context(tc.tile_pool(name="ohs", bufs=1))
        outp = ctx.enter_context(tc.tile_pool(name="outp", bufs=4))
        warm = ctx.enter_context(tc.tile_pool(name="warm", bufs=1))
        # PSUM budget (8 banks): proj/final 2 x [128,512] = 2, scores
        # 2 x [128,1024] = 4, Ohat accumulator 1 x [128,1024] = 2.
        psp = ctx.enter_context(tc.tile_pool(name="psp", bufs=2, space="PSUM"))
        psst = ctx.enter_context(tc.tile_pool(name="psst", bufs=2, space="PSUM"))
        psoh = ctx.enter_context(tc.tile_pool(name="psoh", bufs=1, space="PSUM"))

        # ---- DMA loads.  Column-chunked [128, 1024] transfers (one 2D
        # descriptor each, 2KB per partition line).  Each dma_start costs
        # ~650ns of issue time on its engine's queue, so the issues are
        # spread across engines: sync takes the critical-path Q/K/V order,
        # gpsimd (otherwise idle) takes the weights + tail rows, vector
        # takes one K chunk after its memset.
        wp_hi = const.tile([P, 3 * R], bf)
        wp_lo = const.tile([DLO, 3 * R], bf)
        xall_hi = xin.tile([P, 3 * N], bf)
        xall_lo = xin.tile([DLO, 3 * N], bf)
        rmat_s = const.tile([R, EV], bf)
        # Transfer priority: per-partition DMA write bw is ~1.2-2GB/s, so
        # what matters is bytes-per-partition ahead of each need.  The
        # 4-partition tail rows are extra slow and serialize with each
        # other, so they are split per-half and fronted.  sync carries the
        # 128-partition chunks in first-use order; scalar (idle pre-stream)
        # carries the weights + tail rows.
        nc.sync.dma_start(
            out=xall_hi[:, QOFF:QOFF + HW], in_=xall[0:P, QOFF:QOFF + HW]
        )
        nc.sync.dma_start(
            out=xall_hi[:, KOFF:KOFF + 512], in_=xall[0:P, KOFF:KOFF + 512]
        )
        nc.sync.dma_start(
            out=xall_hi[:, KOFF + 512:KOFF + HW],
            in_=xall[0:P, KOFF + 512:KOFF + HW],
        )
        for lo, hi in (
            (VOFF, VOFF + HW),           # V blocks 0-7
            (VOFF + HW, VOFF + N),       # V blocks 8-15
            (QOFF + HW, QOFF + N),       # Q half 1
        ):
            nc.sync.dma_start(
                out=xall_hi[:, lo:hi], in_=xall[0:P, lo:hi]
            )
        wsrc = warm.tile([P, 512], bf)
        nc.vector.memset(wsrc, 0)
        wdum = warm.tile([P, 4], bf)
        nc.scalar.dma_start(out=wp_hi, in_=wpack[0:P, :])
        nc.scalar.dma_start(out=wp_lo, in_=wpack[P:DP, :])
        nc.scalar.dma_start(
            out=xall_lo[:, QOFF:QOFF + HW], in_=xall[P:DP, QOFF:QOFF + HW]
        )
        nc.scalar.dma_start(
            out=xall_lo[:, KOFF:KOFF + HW], in_=xall[P:DP, KOFF:KOFF + HW]
        )
        nc.scalar.activation(wdum, wsrc[:, 0:4], EXP)
        nc.scalar.dma_start(
            out=xall_lo[:, VOFF:VOFF + HW], in_=xall[P:DP, VOFF:VOFF + HW]
        )
        nc.scalar.dma_start(
            out=xall_lo[:, KOFF + HW:KOFF + N], in_=xall[P:DP, KOFF + HW:KOFF + N]
        )
        nc.scalar.dma_start(
            out=xall_lo[:, VOFF + HW:VOFF + N], in_=xall[P:DP, VOFF + HW:VOFF + N]
        )
        nc.scalar.dma_start(
            out=xall_lo[:, QOFF + HW:QOFF + N], in_=xall[P:DP, QOFF + HW:QOFF + N]
        )
        nc.scalar.dma_start(
            out=xall_hi[:, KOFF + HW:KOFF + N], in_=xall[0:P, KOFF + HW:KOFF + N]
        )
        nc.scalar.dma_start(out=rmat_s, in_=rmat[:, :])

        # ---- junk-matmul burst: keeps the PE clock ramping while the
        # input DMAs land (more junk is interleaved into the projection
        # chain below to fill its DMA-wait bubbles).
        for w in range(3):
            pw = psst.tile([P, HW], f32, tag="pst", name="pw")
            nc.tensor.matmul(pw[:, 0:512], wsrc[:, 0:P], wsrc, start=True, stop=True)

        def junk(n=1):
            for w in range(n):
                pw = psst.tile([P, HW], f32, tag="pst", name="pwj")
                nc.tensor.matmul(
                    pw[:, 0:512], wsrc[:, 0:P], wsrc, start=True, stop=True
                )

        # ---- projection tiles.  QT merged per half so each S_j is ONE
        # [128,1024] matmul; KT chunked [128,512]; VL as fp8 pair tiles.
        qth = [proj.tile([P, HW], bf, tag=f"qh{h}", name=f"qh{h}") for h in range(NH)]
        kts = [proj.tile([P, 512], bf, tag=f"kt{c}", name=f"kt{c}") for c in range(4)]
        vps = [vpool.tile([P, 2, P], f8, tag=f"vp{g}", name=f"vp{g}")
               for g in range(NPAIR)]

        def qk_chunk(dst, woff, xoff, c, on_psst=False, jmid=0):
            if on_psst:
                ppt = psst.tile([P, HW], f32, tag="pst", name="ppk")
                pp = ppt[:, 0:512]
            else:
                pp = psp.tile([P, 512], f32, tag="pp", name="pp")
            nc.tensor.matmul(
                pp,
                wp_hi[:, woff:woff + R],
                xall_hi[:, xoff + c * 512: xoff + (c + 1) * 512],
                start=True,
                stop=False,
            )
            junk(jmid)
            nc.tensor.matmul(
                pp,
                wp_lo[:, woff:woff + R],
                xall_lo[:, xoff + c * 512: xoff + (c + 1) * 512],
                start=False,
                stop=True,
            )
            nc.vector.tensor_copy(dst, pp)

        def vl_group(g4):
            """Project VL for j = 4*g4 .. 4*g4+3 into fp8 pair tiles."""
            pv = psp.tile([P, 512], f32, tag="pp", name="pv")
            for t in range(4):
                j = 4 * g4 + t
                nc.tensor.matmul(
                    pv[:, ts(t, P)],
                    xall_hi[:, VOFF + j * P: VOFF + (j + 1) * P],
                    wp_hi[:, LOFF:LOFF + R],
                    start=True,
                    stop=False,
                )
                nc.tensor.matmul(
                    pv[:, ts(t, P)],
                    xall_lo[:, VOFF + j * P: VOFF + (j + 1) * P],
                    wp_lo[:, LOFF:LOFF + R],
                    start=False,
                    stop=True,
                )
            for t in range(2):
                g = 2 * g4 + t
                nc.vector.tensor_copy(vps[g][:, 0, :], pv[:, ts(2 * t, P)])
                nc.vector.tensor_copy(vps[g][:, 1, :], pv[:, ts(2 * t + 1, P)])

        def s_exp(h, j, edst):
            """S^T_j for half h (512-col matmuls: PSUM-bank limit), exp (fp8)."""
            pst = psst.tile([P, HW], f32, tag="pst", name="pst")
            for c in range(2):
                nc.tensor.matmul(
                    pst[:, ts(c, 512)],
                    kts[j // 4][:, ts(j % 4, P)],
                    qth[h][:, ts(c, 512)],
                    start=True,
                    stop=True,
                )
            nc.scalar.activation(edst, pst, EXP)

        def o_pair(poh, g, ep):
            """DoubleRow fp8 matmuls: contract j-blocks 2g and 2g+1 at once."""
            for c in range(2):
                nc.tensor.matmul(
                    poh[:, ts(c, 512)],
                    vps[g],
                    ep[:, :, ts(c, 512)],
                    start=(g == 0),
                    stop=(g == NPAIR - 1),
                    perf_mode=DR,
                )

        def finalize_group(h, g, ohat, act_mul=False):
            """Two i-blocks -> O' = Ohat R^T, normalize, DMA out.  act_mul
            puts one of the two normalizes on ACT and the output DMA issue
            on the scalar queue (post-exp-stream only)."""
            stage = outp.tile([P, 2, D], f32, tag="stage", name="stage")
            for t in range(2):
                i = 2 * g + t
                po = psp.tile([P, EV], f32, tag="pp", name="po")
                nc.tensor.matmul(
                    po, ohat[:, ts(i % 8, P)], rmat_s, start=True, stop=True
                )
                rec = outp.tile([P, 1], f32, tag="rec", name="rec")
                nc.vector.reciprocal(rec, po[:, D:D + 1])
                if act_mul and t == 1:
                    nc.scalar.activation(
                        stage[:, t, :], po[:, 0:D], COPY, scale=rec
                    )
                else:
                    nc.vector.tensor_scalar_mul(stage[:, t, :], po[:, 0:D], rec)
            nc.sync.dma_start(
                out=out[g * 256:(g + 1) * 256, :].rearrange(
                    "(t p) e -> p t e", p=P
                ),
                in_=stage,
            )

        # ---- h=0 stream: project what each j needs just in time, start
        # the exp stream as early as possible, trail it with the DoubleRow
        # O accumulation; VL groups fill PE slack between S matmuls.
        qk_chunk(qth[0][:, 0:512], AQOFF, QOFF, 0, jmid=1)
        qk_chunk(qth[0][:, 512:HW], AQOFF, QOFF, 1)
        qk_chunk(kts[0], AKOFF, KOFF, 0, on_psst=True, jmid=1)

        poh0 = psoh.tile([P, HW], f32, tag="poh", name="poh0")
        eps0 = []

        def h0_step(j):
            if j % 2 == 0:
                ep = epool.tile([P, 2, HW], f8, tag="e", name=f"e0_{j // 2}")
                eps0.append(ep)
            s_exp(0, j, eps0[j // 2][:, j % 2, :])

        h0_step(0)
        h0_step(1)
        qk_chunk(kts[1], AKOFF, KOFF, 1)
        h0_step(2)
        h0_step(3)
        vl_group(0)
        h0_step(4)
        o_pair(poh0, 0, eps0[0])
        h0_step(5)
        qk_chunk(kts[2], AKOFF, KOFF, 2)
        h0_step(6)
        o_pair(poh0, 1, eps0[1])
        vl_group(1)
        h0_step(7)
        qk_chunk(kts[3], AKOFF, KOFF, 3)
        h0_step(8)
        o_pair(poh0, 2, eps0[2])
        h0_step(9)
        vl_group(2)
        h0_step(10)
        o_pair(poh0, 3, eps0[3])
        h0_step(11)
        vl_group(3)
        h0_step(12)
        o_pair(poh0, 4, eps0[4])
        h0_step(13)
        qk_chunk(qth[1][:, 0:512], AQOFF, QOFF, 2)
        qk_chunk(qth[1][:, 512:HW], AQOFF, QOFF, 3)
        h0_step(14)
        o_pair(poh0, 5, eps0[5])
        h0_step(15)
        o_pair(poh0, 6, eps0[6])
        o_pair(poh0, 7, eps0[7])
        ohat0 = ohs.tile([P, HW], bf, tag="oh0", name="oh0")
        nc.vector.tensor_copy(ohat0, poh0)

        # ---- h=1 stream with h=0 finalization interleaved.
        poh1 = psoh.tile([P, HW], f32, tag="poh", name="poh1")
        eps1 = []

        def h1_step(j):
            if j % 2 == 0:
                ep = epool.tile([P, 2, HW], f8, tag="e", name=f"e1_{j // 2}")
                eps1.append(ep)
            s_exp(1, j, eps1[j // 2][:, j % 2, :])

        for j in range(NB):
            h1_step(j)
            if j % 2 == 1:
                g = j // 2
                o_pair(poh1, g, eps1[g])
                if g < 4:
                    # h0 finals run mid-stream where DVE is idle
                    finalize_group(0, g, ohat0)
        ohat1 = ohs.tile([P, HW], bf, tag="oh1", name="oh1")
        # ACT is free once the exp stream ends; copy in 256-col pieces
        # alternating ACT/DVE so finalize group g can start on piece g
        for p4 in range(4):
            if p4 % 2 == 0:
                nc.scalar.activation(
                    ohat1[:, ts(p4, 256)], poh1[:, ts(p4, 256)], COPY
                )
            else:
                nc.vector.tensor_copy(ohat1[:, ts(p4, 256)], poh1[:, ts(p4, 256)])
            finalize_group(1, 4 + p4, ohat1, act_mul=True)

    return nc


def dedup_ldweights(nc):
    """Drop Ldweights instructions that reload the exact weights already in
    the PE array (same AP, nothing clobbering in between).  The PE keeps the
    stationary operand across matmuls, so a back-to-back identical reload is
    pure dispatch overhead (~107ns each).  Only sync-free Ldweights are
    dropped so semaphore ordering is untouched."""
    dropped = 0
    for f in nc.m.functions:
        for blk in f.blocks:
            insts = list(blk.instructions)
            kept = []
            last_key = None
            for ins in insts:
                tname = type(ins).__name__
                if "PE" in str(getattr(ins, "engine", "")):
                    if tname == "InstLdweights":
                        ap = ins.ins[0]
                        key = (
                            ap.memref,
                            ap.offset,
                            str(ap.ap),
                            str(ap.dtype),
                            str(getattr(ins, "is_transpose", None)),
                        )
                        si = ins.sync_info
                        no_sync = si is None or (
                            len(si.on_wait) == 0 and len(si.on_update) == 0
                        )
                        if key == last_key and no_sync:
                            dropped += 1
                            continue
                        last_key = key
                    elif tname not in (
                        "InstMatmult",
                        "InstEventSemaphore",
                        "InstNoOp",
                        "InstDrain",
                    ):
                        last_key = None
                kept.append(ins)
            if len(kept) != len(insts):
                blk.instructions = kept
    return dropped


def prep_host(query, key, value, Wq, bq, Wk, bk, Wv, bv):
    """Host-side layout/algebra prep. Returns per-core input maps."""
    s = np.sqrt(np.float64(D))
    Wqp = np.concatenate([Wq, bq[:, None]], axis=1)  # [131, 132]
    Wkp = np.concatenate([Wk, bk[:, None]], axis=1)
    G = (Wqp.astype(np.float64).T @ Wkp.astype(np.float64)) / s  # [132, 132]
    U, S, Vt = np.linalg.svd(G)
    Aq = (U[:, :R] * np.sqrt(S[:R])).astype(np.float32)  # [132, 128]
    Ak = (Vt[:R, :].T * np.sqrt(S[:R])).astype(np.float32)

    W2 = np.zeros((DP, EV), np.float64)  # maps X -> [V | 1]
    W2[:D, :D] = Wv.T
    W2[D, :D] = bv
    W2[D, D] = 1.0
    U2, S2, V2t = np.linalg.svd(W2)
    L = (U2[:, :R] * np.sqrt(S2[:R])).astype(np.float32)  # [132, 128]
    Rm = (V2t[:R, :].T * np.sqrt(S2[:R])).astype(np.float32)  # [132, 128]

    wpack = np.concatenate([Aq, Ak, L], axis=1)  # [132, 384]
    wpack16 = np.ascontiguousarray(wpack.astype(_BF16))
    rmat16 = np.ascontiguousarray(Rm.T.astype(_BF16))  # [128, 132]

    ones_row = np.ones((1, N), np.float32)
    in_maps = []
    for c in range(NCORES):
        xs = [np.concatenate([x.T, ones_row], axis=0)
              for x in (query[c], key[c], value[c])]
        xallc = np.concatenate(xs, axis=1)  # [132, 6144]
        in_maps.append({
            "xall": np.ascontiguousarray(xallc.astype(_BF16)),
            "wpack": wpack16,
            "rmat": rmat16,
        })
    return in_maps


_NC_CACHE = {}


def _get_nc():
    if "nc" not in _NC_CACHE:
        nc = build_nc()
        if not nc.is_finalized():
            nc.finalize()  # Bacc.finalize runs the wait-split/EVSEM passes
        dedup_ldweights(nc)
        _NC_CACHE["nc"] = nc
    return _NC_CACHE["nc"]


def run_on_cores(in_maps, trace=False, **kw):
    from concourse.bass_utils import run_bass_kernel_spmd

    nc = _get_nc()
    return run_bass_kernel_spmd(nc, in_maps, core_ids=list(range(NCORES)),
                                trace=trace, **kw)


def kernel(query, key, value, Wq, bq, Wk, bk, Wv, bv):
    in_maps = prep_host(query, key, value, Wq, bq, Wk, bk, Wv, bv)
    res = run_on_cores(in_maps)
    return np.stack([np.asarray(res.results[c]["out"]) for c in range(NCORES)])


# revision 27
# speedup vs baseline: 2.3563x; 1.2238x over previous
"""Trainium2 Bass kernel for batched self-attention with input projections.

Problem: B=8, N=2048, D=131
    Q = q @ Wq.T + bq;  K = k @ Wk.T + bk;  V = v @ Wv.T + bv
    out = softmax(Q K^T / sqrt(131)) V

One batch element per NeuronCore (8 cores, no communication).

Host prep (layout/algebra only):
  - Tokens augmented with a ones-row: X = [x^T; 1] in [132, 2048] so biases
    fold into the projection matmuls.
  - Scores: Q K^T = Xq (Wq'^T Wk'/sqrt(D)) Xk^T = Xq G Xk^T, G [132,132].
    SVD-truncate G to rank 128 (exact rank 131; error ~2e-5) so the big S
    matmul is a single K=128 contraction:  S = (Xq Aq)(Xk Ak)^T.
  - Value path: W2 [132,132] maps X -> [V | 1] (bias row + denominator
    ones-column).  SVD-truncate W2 = L R^T to rank 128 so the O-matmul
    contracts into a 128-wide latent:  O' = (P Xv L) R^T, with O'[:,131]
    the softmax denominator.
  - Projections/S in bf16 (fp32 PSUM accumulation); the O path (exp
    weights E and the value latent VL) in fp8e4m3 so the O accumulation
    runs in DoubleRow perf mode: each matmul contracts TWO 128-token
    k-tiles at once (lhsT [128,2,128], rhs [128,2,1024]), halving the PE
    time of the biggest accumulation.  Measured rel err ~1.05e-2 (numpy
    bit-accurate sim of the dtype pipeline).  |S| < 3 so softmax without
    max-subtraction is safe.

Per core:
  QT[e',n] = Aq^T Xq, KT[e',n] = Ak^T Xk      (hi[128]+lo[4] d-chunks)
  VL[n,l]  = Xv^T L -> fp8 pair tiles [128,2,128] (j, j+1)
  for i-half h (1024 cols), j-block (16):
      ST = KT_j^T QT_h  (one [128,1024] matmul) -> exp on ACT -> fp8 E
      pair tiles [128,2,1024]; every 2 j's one DoubleRow matmul
      accumulates Ohat^T[l, h] in PSUM.
  O'[i,132] = Ohat_i R^T;  out = O'[:,0:131] / O'[:,131]

ACT does exp only (the activation table preloads via a dummy exp during
the DMA window); DVE does all PSUM->SBUF copies and the normalize; a
burst of junk matmuls during the input DMA warms the PE clock gate; a
post-finalize pass drops Ldweights instructions that reload identical
weights.
"""

import numpy as np
import ml_dtypes

P = 128          # partitions / PE width
N = 2048         # tokens per core
D = 131          # embed dim
DP = 132         # embed dim + ones row
DLO = DP - P     # tail contraction rows (4)
R = 128          # truncated rank (QK interaction and V latent)
EV = 132         # final output cols (131 + denominator)
NB = N // P      # 16 token blocks
NPAIR = NB // 2  # 8 j-block pairs (DoubleRow granule)
HW = 1024        # i-half width
NH = N // HW     # 2 halves
NCORES = 8

QOFF, KOFF, VOFF = 0, N, 2 * N          # column offsets in packed xall
AQOFF, AKOFF, LOFF = 0, R, 2 * R        # column offsets in packed weights

_BF16 = ml_dtypes.bfloat16


def build_nc():
    """Build the single-core Bass graph (same NEFF runs SPMD on all 8 cores)."""
    from contextlib import ExitStack

    import concourse.bacc as bacc
    import concourse.mybir as mybir
    import concourse.tile as tile
    from concourse.bass import ts

    bf = mybir.dt.bfloat16
    f8 = mybir.dt.float8e4
    f32 = mybir.dt.float32
    EXP = mybir.ActivationFunctionType.Exp
    COPY = mybir.ActivationFunctionType.Copy
    DR = mybir.MatmulPerfMode.DoubleRow

    nc = bacc.Bacc()
    xall = nc.declare_dram_parameter("xall", [DP, 3 * N], bf, isOutput=False)
    wpack = nc.declare_dram_parameter("wpack", [DP, 3 * R], bf, isOutput=False)
    rmat = nc.declare_dram_parameter("rmat", [R, EV], bf, isOutput=False)
    out = nc.declare_dram_parameter("out", [N, D], f32, isOutput=True)

    with tile.TileContext(nc) as tc, ExitStack() as ctx:
        const = ctx.enter_context(tc.tile_pool(name="const", bufs=1))
        xin = ctx.enter_context(tc.tile_pool(name="xin", bufs=1))
        proj = ctx.enter_context(tc.tile_pool(name="proj", bufs=1))
        vpool = ctx.enter_context(tc.tile_pool(name="vpool", bufs=1))
        epool = ctx.enter_context(tc.tile_pool(name="epool", bufs=6))
        ebp = ctx.enter_context(tc.tile_pool(name="ebp", bufs=4))
        ohs = ctx.enter_context(tc.tile_pool(name="ohs", bufs=1))
        outp = ctx.enter_context(tc.tile_pool(name="outp", bufs=4))
        warm = ctx.enter_context(tc.tile_pool(name="warm", bufs=1))
        # PSUM budget (8 banks): proj/final 2 x [128,512] = 2, scores
        # 2 x [128,1024] = 4, Ohat accumulator 1 x [128,1024] = 2.
        psp = ctx.enter_context(tc.tile_pool(name="psp", bufs=2, space="PSUM"))
        psst = ctx.enter_context(tc.tile_pool(name="psst", bufs=2, space="PSUM"))
        psoh = ctx.enter_context(tc.tile_pool(name="psoh", bufs=1, space="PSUM"))

        # ---- DMA loads.  Column-chunked [128, 1024] transfers (one 2D
        # descriptor each, 2KB per partition line).  Each dma_start costs
        # ~650ns of issue time on its engine's queue, so the issues are
        # spread across engines: sync takes the critical-path Q/K/V order,
        # gpsimd (otherwise idle) takes the weights + tail rows, vector
        # takes one K chunk after its memset.
        wp_hi = const.tile([P, 3 * R], bf)
        wp_lo = const.tile([DLO, 3 * R], bf)
        xall_hi = xin.tile([P, 3 * N], bf)
        xall_lo = xin.tile([DLO, 3 * N], bf)
        rmat_s = const.tile([R, EV], bf)
        # Transfer priority: per-partition DMA write bw is ~1.2-2GB/s, so
        # what matters is bytes-per-partition ahead of each need.  The
        # 4-partition tail rows are extra slow and serialize with each
        # other, so they are split per-half and fronted.  sync carries the
        # 128-partition chunks in first-use order; scalar (idle pre-stream)
        # carries the weights + tail rows.
        nc.sync.dma_start(
            out=xall_hi[:, QOFF:QOFF + HW], in_=xall[0:P, QOFF:QOFF + HW]
        )
        nc.sync.dma_start(
            out=xall_hi[:, KOFF:KOFF + 512], in_=xall[0:P, KOFF:KOFF + 512]
        )
        nc.sync.dma_start(
            out=xall_hi[:, KOFF + 512:KOFF + HW],
            in_=xall[0:P, KOFF + 512:KOFF + HW],
        )
        for lo, hi in (
            (VOFF, VOFF + HW),           # V blocks 0-7
            (VOFF + HW, VOFF + N),       # V blocks 8-15
            (KOFF + HW, KOFF + N),       # K chunks 2-3
            (QOFF + HW, QOFF + N),       # Q half 1
        ):
            nc.sync.dma_start(
                out=xall_hi[:, lo:hi], in_=xall[0:P, lo:hi]
            )
        wsrc = warm.tile([P, 512], bf)
        nc.vector.memset(wsrc, 0)
        wdum = warm.tile([P, 4], bf)
        # scalar carries only the first-exp-critical small transfers so the
        # exp stream is not stuck behind DMA issues on the ACT queue
        nc.scalar.dma_start(out=wp_hi, in_=wpack[0:P, :])
        nc.scalar.dma_start(out=wp_lo, in_=wpack[P:DP, :])
        nc.scalar.dma_start(
            out=xall_lo[:, QOFF:QOFF + HW], in_=xall[P:DP, QOFF:QOFF + HW]
        )
        nc.scalar.dma_start(
            out=xall_lo[:, KOFF:KOFF + HW], in_=xall[P:DP, KOFF:KOFF + HW]
        )
        nc.scalar.activation(wdum, wsrc[:, 0:4], EXP)
        # remaining tail rows + rmat ride the otherwise-idle gpsimd queue
        nc.gpsimd.dma_start(
            out=xall_lo[:, VOFF:VOFF + HW], in_=xall[P:DP, VOFF:VOFF + HW]
        )
        nc.gpsimd.dma_start(
            out=xall_lo[:, KOFF + HW:KOFF + N], in_=xall[P:DP, KOFF + HW:KOFF + N]
        )
        nc.gpsimd.dma_start(
            out=xall_lo[:, VOFF + HW:VOFF + N], in_=xall[P:DP, VOFF + HW:VOFF + N]
        )
        nc.gpsimd.dma_start(
            out=xall_lo[:, QOFF + HW:QOFF + N], in_=xall[P:DP, QOFF + HW:QOFF + N]
        )
        nc.gpsimd.dma_start(out=rmat_s, in_=rmat[:, :])

        # ---- junk-matmul burst: keeps the PE clock ramping while the
        # input DMAs land (more junk is interleaved into the projection
        # chain below to fill its DMA-wait bubbles).
        for w in range(3):
            pw = psst.tile([P, HW], f32, tag="pst", name="pw")
            nc.tensor.matmul(pw[:, 0:512], wsrc[:, 0:P], wsrc, start=True, stop=True)

        def junk(n=1):
            for w in range(n):
                pw = psst.tile([P, HW], f32, tag="pst", name="pwj")
                nc.tensor.matmul(
                    pw[:, 0:512], wsrc[:, 0:P], wsrc, start=True, stop=True
                )

        # ---- projection tiles.  QT merged per half so each S_j is ONE
        # [128,1024] matmul; KT chunked [128,512]; VL as fp8 pair tiles.
        qth = [proj.tile([P, HW], bf, tag=f"qh{h}", name=f"qh{h}") for h in range(NH)]
        kts = [proj.tile([P, 512], bf, tag=f"kt{c}", name=f"kt{c}") for c in range(4)]
        vps = [vpool.tile([P, 2, P], f8, tag=f"vp{g}", name=f"vp{g}")
               for g in range(NPAIR)]

        def qk_chunk(dst, woff, xoff, c, on_psst=False, jmid=0):
            if on_psst:
                ppt = psst.tile([P, HW], f32, tag="pst", name="ppk")
                pp = ppt[:, 0:512]
            else:
                pp = psp.tile([P, 512], f32, tag="pp", name="pp")
            nc.tensor.matmul(
                pp,
                wp_hi[:, woff:woff + R],
                xall_hi[:, xoff + c * 512: xoff + (c + 1) * 512],
                start=True,
                stop=False,
            )
            junk(jmid)
            nc.tensor.matmul(
                pp,
                wp_lo[:, woff:woff + R],
                xall_lo[:, xoff + c * 512: xoff + (c + 1) * 512],
                start=False,
                stop=True,
            )
            nc.vector.tensor_copy(dst, pp)

        def vl_group(g4):
            """Project VL for j = 4*g4 .. 4*g4+3 into fp8 pair tiles."""
            pv = psp.tile([P, 512], f32, tag="pp", name="pv")
            for t in range(4):
                j = 4 * g4 + t
                nc.tensor.matmul(
                    pv[:, ts(t, P)],
                    xall_hi[:, VOFF + j * P: VOFF + (j + 1) * P],
                    wp_hi[:, LOFF:LOFF + R],
                    start=True,
                    stop=False,
                )
                nc.tensor.matmul(
                    pv[:, ts(t, P)],
                    xall_lo[:, VOFF + j * P: VOFF + (j + 1) * P],
                    wp_lo[:, LOFF:LOFF + R],
                    start=False,
                    stop=True,
                )
            for t in range(2):
                g = 2 * g4 + t
                nc.vector.tensor_copy(vps[g][:, 0, :], pv[:, ts(2 * t, P)])
                nc.vector.tensor_copy(vps[g][:, 1, :], pv[:, ts(2 * t + 1, P)])

        def s_exp(h, j, edst):
            """S^T_j for half h (512-col matmuls: PSUM-bank limit), exp (fp8)."""
            pst = psst.tile([P, HW], f32, tag="pst", name="pst")
            for c in range(2):
                nc.tensor.matmul(
                    pst[:, ts(c, 512)],
                    kts[j // 4][:, ts(j % 4, P)],
                    qth[h][:, ts(c, 512)],
                    start=True,
                    stop=True,
                )
            nc.scalar.activation(edst, pst, EXP)

        def o_pair(poh, g, ep):
            """DoubleRow fp8 matmuls: contract j-blocks 2g and 2g+1 at once."""
            for c in range(2):
                nc.tensor.matmul(
                    poh[:, ts(c, 512)],
                    vps[g],
                    ep[:, :, ts(c, 512)],
                    start=(g == 0),
                    stop=(g == NPAIR - 1),
                    perf_mode=DR,
                )

        def finalize_group(h, g, ohat, act_mul=False):
            """Two i-blocks -> O' = Ohat R^T, normalize, DMA out.  act_mul
            puts one of the two normalizes on ACT and the output DMA issue
            on the scalar queue (post-exp-stream only)."""
            stage = outp.tile([P, 2, D], f32, tag="stage", name="stage")
            for t in range(2):
                i = 2 * g + t
                po = psp.tile([P, EV], f32, tag="pp", name="po")
                nc.tensor.matmul(
                    po, ohat[:, ts(i % 8, P)], rmat_s, start=True, stop=True
                )
                rec = outp.tile([P, 1], f32, tag="rec", name="rec")
                nc.vector.reciprocal(rec, po[:, D:D + 1])
                if act_mul and t == 1:
                    nc.scalar.activation(
                        stage[:, t, :], po[:, 0:D], COPY, scale=rec
                    )
                else:
                    nc.vector.tensor_scalar_mul(stage[:, t, :], po[:, 0:D], rec)
            nc.sync.dma_start(
                out=out[g * 256:(g + 1) * 256, :].rearrange(
                    "(t p) e -> p t e", p=P
                ),
                in_=stage,
            )

        # ---- h=0 stream: project what each j needs just in time, start
        # the exp stream as early as possible, trail it with the DoubleRow
        # O accumulation; VL groups fill PE slack between S matmuls.
        qk_chunk(qth[0][:, 0:512], AQOFF, QOFF, 0, jmid=1)
        qk_chunk(qth[0][:, 512:HW], AQOFF, QOFF, 1)
        qk_chunk(kts[0], AKOFF, KOFF, 0, on_psst=True, jmid=1)

        poh0 = psoh.tile([P, HW], f32, tag="poh", name="poh0")
        eps0 = []

        def h0_step(j):
            if j % 2 == 0:
                ep = epool.tile([P, 2, HW], f8, tag="e", name=f"e0_{j // 2}")
                eps0.append(ep)
            s_exp(0, j, eps0[j // 2][:, j % 2, :])

        h0_step(0)
        h0_step(1)
        qk_chunk(kts[1], AKOFF, KOFF, 1)
        h0_step(2)
        h0_step(3)
        vl_group(0)
        h0_step(4)
        o_pair(poh0, 0, eps0[0])
        h0_step(5)
        qk_chunk(kts[2], AKOFF, KOFF, 2)
        h0_step(6)
        o_pair(poh0, 1, eps0[1])
        vl_group(1)
        h0_step(7)
        qk_chunk(kts[3], AKOFF, KOFF, 3)
        h0_step(8)
        o_pair(poh0, 2, eps0[2])
        h0_step(9)
        vl_group(2)
        h0_step(10)
        o_pair(poh0, 3, eps0[3])
        h0_step(11)
        vl_group(3)
        h0_step(12)
        o_pair(poh0, 4, eps0[4])
        h0_step(13)
        qk_chunk(qth[1][:, 0:512], AQOFF, QOFF, 2)
        qk_chunk(qth[1][:, 512:HW], AQOFF, QOFF, 3)
        h0_step(14)
        o_pair(poh0, 5, eps0[5])
        h0_step(15)
        o_pair(poh0, 6, eps0[6])
        o_pair(poh0, 7, eps0[7])
        ohat0 = ohs.tile([P, HW], bf, tag="oh0", name="oh0")
        nc.vector.tensor_copy(ohat0, poh0)

        # ---- h=1 stream with h=0 finalization interleaved.
        poh1 = psoh.tile([P, HW], f32, tag="poh", name="poh1")
        eps1 = []

        def h1_step(j):
            if j % 2 == 0:
                ep = epool.tile([P, 2, HW], f8, tag="e", name=f"e1_{j // 2}")
                eps1.append(ep)
            s_exp(1, j, eps1[j // 2][:, j % 2, :])

        for j in range(NB):
            h1_step(j)
            if j % 2 == 1:
                g = j // 2
                o_pair(poh1, g, eps1[g])
                if g < 4:
                    # h0 finals run mid-stream where DVE is idle
                    finalize_group(0, g, ohat0)
        ohat1 = ohs.tile([P, HW], bf, tag="oh1", name="oh1")
        # ACT is free once the exp stream ends; copy in 256-col pieces
        # alternating ACT/DVE so finalize group g can start on piece g
        for p4 in range(4):
            if p4 % 2 == 0:
                nc.scalar.activation(
                    ohat1[:, ts(p4, 256)], poh1[:, ts(p4, 256)], COPY
                )
            else:
                nc.vector.tensor_copy(ohat1[:, ts(p4, 256)], poh1[:, ts(p4, 256)])
            finalize_group(1, 4 + p4, ohat1, act_mul=True)

    return nc


def dedup_ldweights(nc):
    """Drop Ldweights instructions that reload the exact weights already in
    the PE array (same AP, nothing clobbering in between).  The PE keeps the
    stationary operand across matmuls, so a back-to-back identical reload is
    pure dispatch overhead (~107ns each).  Only sync-free Ldweights are
    dropped so semaphore ordering is untouched."""
    dropped = 0
    for f in nc.m.functions:
        for blk in f.blocks:
            insts = list(blk.instructions)
            kept = []
            last_key = None
            for ins in insts:
                tname = type(ins).__name__
                if "PE" in str(getattr(ins, "engine", "")):
                    if tname == "InstLdweights":
                        ap = ins.ins[0]
                        key = (
                            ap.memref,
                            ap.offset,
                            str(ap.ap),
                            str(ap.dtype),
                            str(getattr(ins, "is_transpose", None)),
                        )
                        si = ins.sync_info
                        no_sync = si is None or (
                            len(si.on_wait) == 0 and len(si.on_update) == 0
                        )
                        if key == last_key and no_sync:
                            dropped += 1
                            continue
                        last_key = key
                    elif tname not in (
                        "InstMatmult",
                        "InstEventSemaphore",
                        "InstNoOp",
                        "InstDrain",
                    ):
                        last_key = None
                kept.append(ins)
            if len(kept) != len(insts):
                blk.instructions = kept
    return dropped


def prep_host(query, key, value, Wq, bq, Wk, bk, Wv, bv):
    """Host-side layout/algebra prep. Returns per-core input maps."""
    s = np.sqrt(np.float64(D))
    Wqp = np.concatenate([Wq, bq[:, None]], axis=1)  # [131, 132]
    Wkp = np.concatenate([Wk, bk[:, None]], axis=1)
    G = (Wqp.astype(np.float64).T @ Wkp.astype(np.float64)) / s  # [132, 132]
    U, S, Vt = np.linalg.svd(G)
    Aq = (U[:, :R] * np.sqrt(S[:R])).astype(np.float32)  # [132, 128]
    Ak = (Vt[:R, :].T * np.sqrt(S[:R])).astype(np.float32)

    W2 = np.zeros((DP, EV), np.float64)  # maps X -> [V | 1]
    W2[:D, :D] = Wv.T
    W2[D, :D] = bv
    W2[D, D] = 1.0
    U2, S2, V2t = np.linalg.svd(W2)
    L = (U2[:, :R] * np.sqrt(S2[:R])).astype(np.float32)  # [132, 128]
    Rm = (V2t[:R, :].T * np.sqrt(S2[:R])).astype(np.float32)  # [132, 128]

    wpack = np.concatenate([Aq, Ak, L], axis=1)  # [132, 384]
    wpack16 = np.ascontiguousarray(wpack.astype(_BF16))
    rmat16 = np.ascontiguousarray(Rm.T.astype(_BF16))  # [128, 132]

    ones_row = np.ones((1, N), np.float32)
    in_maps = []
    for c in range(NCORES):
        xs = [np.concatenate([x.T, ones_row], axis=0)
              for x in (query[c], key[c], value[c])]
        xallc = np.concatenate(xs, axis=1)  # [132, 6144]
        in_maps.append({
            "xall": np.ascontiguousarray(xallc.astype(_BF16)),
            "wpack": wpack16,
            "rmat": rmat16,
        })
    return in_maps


_NC_CACHE = {}


def _get_nc():
    if "nc" not in _NC_CACHE:
        nc = build_nc()
        if not nc.is_finalized():
            nc.finalize()  # Bacc.finalize runs the wait-split/EVSEM passes
        dedup_ldweights(nc)
        _NC_CACHE["nc"] = nc
    return _NC_CACHE["nc"]


def run_on_cores(in_maps, trace=False, **kw):
    from concourse.bass_utils import run_bass_kernel_spmd

    nc = _get_nc()
    return run_bass_kernel_spmd(nc, in_maps, core_ids=list(range(NCORES)),
                                trace=trace, **kw)


def kernel(query, key, value, Wq, bq, Wk, bk, Wv, bv):
    in_maps = prep_host(query, key, value, Wq, bq, Wk, bk, Wv, bv)
    res = run_on_cores(in_maps)
    return np.stack([np.asarray(res.results[c]["out"]) for c in range(NCORES)])
